# revision 34
# baseline (speedup 1.0000x reference)
"""Trainium2 Bass kernel for nn_Attention_43542378447097.

GroupNorm -> multi-head causal self-attention -> out-proj, then the
reference's broadcast add:

    out(B,S,C) + residual(B,C,1,C)  ->  (B,C,S,C)   [right-aligned numpy
    broadcasting, so batches MIX]:

    result[i, j, k, l] = A[j, k, l] + xn[i, j, l]

where A[j] = attention output of batch j and xn[i] = groupnorm output of
batch i.  Output is (96, 96, 96, 96) (~340 MB fp32) -> memory bound.
Device computes/stores bf16 (harness gate is global rel err < 2e-2; bf16
costs ~5e-3) and the host upcasts, so each core writes ~21 MB.

Sharding: core c owns batches/rows i in [12c, 12c+12).
  Phase 1 (local): batched groupnorm + attention -> A_local (12,96,96) bf16
  Phase 2: AllGather A_local over 8 cores -> A_full (96,96,96) bf16
  Phase 3 (per local i): result[i] = A_full + (xn_i + bo) broadcast over k,
    bf16 adds on DVE (2x_1p mode), one full-row 1.77 MB store per i with
    the store queue alternating between the SP and ACT DMA rings.

Attention algebra (all matmuls bf16, 1 PE cycle/row vs 4 for fp32):
  xn_e = [xn, 1] (ones row appended to xnT); host precomputes per head
    Msc_h = [Wq_h;bq_h] [Wk_h;bk_h]^T / sqrt(dk)   (97x97)
    Nv_h  = [Wv_h;bv_h] Wo_h                       (97x96)
  R_h      = matmul(lhsT=Msc_h, rhs=xn_eT)      (8 per batch)
  scoresT  = matmul(lhsT=xn_eT, rhs=R 4-head pack)  [sk, (h sq)] -- the
             shared stationary xn_eT lets 4 heads ride one 384-wide rhs,
             and softmax sums run over the partition dim via a ones-matmul
  exp on ACT -> causal mask multiply -> denominators via ones-matmul ->
  reciprocal -> attnT normalize (DVE)
  vw       = matmul(lhsT=xn_eT, rhs=Nv 4-head pack) (= v_h @ Wo_h rows)
  A        = sum_h matmul(lhsT=attnT_h, rhs=vw_h)   (+ bo via phase 3)
This removes separate q/k/v/out-proj evictions: per batch only R, exp,
vw, attn and one A eviction touch PSUM.  GroupNorm stats are batched
across all 12 local batches (one wide partition-major x DMA, two big
row-reduces + one tiny fp32 matmul through the group-averaging mask);
rstd is an all-DVE Newton rsqrt, so ACT only ever loads the
exp_and_others table set (Exp/Square).

The bench substitute gather is chunked in QB-batch pieces emitted right
after their producing pipeline groups, so all but the last chunk overlap
remaining phase-1 compute (the real-collective path keeps a single
AllGather: Shared DRAM allows one writer).
"""

import sys

sys.path.insert(0, "/opt/trn_rl_repo")

import numpy as np

B_TOTAL = 96
C = 96
S = 96
NH = 8
DK = 96
G = 8
NCORES = 8
BPC = B_TOTAL // NCORES  # 12
EPS = 1e-5
NFREE = S * C  # 9216
HALFN = NFREE // 2  # assembly half-slab width
# assembly half-slabs 0..23 interleaved between VectorE (19) and GpSimd (5):
# with bf16 operands DVE runs 2x (2x_1p mode) at ~2.4us/slab while GpSimd
# adds run at 0.42 efficiency (~9.1us/slab), so DVE takes most of them.
_GPSIMD_HALVES = frozenset({2, 7, 12, 17, 22})

_PROG = None

# engine-assignment knobs (HW bisection; see bench_knobs.py)
POOL_MASK = False  # causal mask multiply on GpSimd (else DVE)
POOL_ATTN = False  # attn normalize multiply on GpSimd (else DVE)
ALOC_ENG = "sync"  # a_loc store queue: gpsimd | scalar | sync
OUT_SPREAD = 1  # output store queues: 0=sync, 1=3-way, 2=sync/scalar
QB = 6  # gather chunk size in batches (3, 4, 6, or 12)
ILV = 3  # software-pipeline interleave width (batches per group)
FULL_SLAB = True  # phase-3: one full-row store per i (else two halves)
LOADS_Q = False  # a_sb loads in k-quarters alternating queues
PEFILL = 0  # filler matmuls per stage to hold the PE clock up (0=off)
R2ACT = False  # R pack #2 eviction on ACT (else DVE)
POOL_IS = 0  # with FULL_SLAB: number of i rows whose adds run on GpSimd
GATHER_SPREAD = 0  # 0=sync only, 1=spread last chunk, 2=spread all chunks
ASYM = False  # gather chunks {0-8}{9-11}: small exposed tail
RES_BUFS = 3  # phase-3 result tile ring depth
DEEP = False  # +1 on all phase-1 work-tile rings (latency hedging)
OUT_ROT = 0  # rotation offset of the 3-way store queue ring
LOAD2_SCALAR = False  # second a_sb load on ACT queue (else sync)
PS_BUFS = 4  # psum bufs for the shared matmul tag
PXT_BUFS = 2  # psum bufs for the transpose tag (PS_BUFS+PXT_BUFS+2 <= 8)


def _build_program(skip_collective=False, loop_n=1, phases="123"):
    import contextlib

    import concourse.bass as bass
    import concourse.tile as tile
    from concourse import bacc, mybir

    f32 = mybir.dt.float32
    bf16 = mybir.dt.bfloat16
    AF = mybir.ActivationFunctionType
    ALU = mybir.AluOpType
    AX = mybir.AxisListType

    nc = bacc.Bacc(
        "TRN2",
        target_bir_lowering=False,
        debug=False,
        enable_asserts=False,
        num_devices=NCORES,
    )

    # x arrives partition-major (p, b, w) so one wide DMA loads all
    # 12 batches with 4.6 KB contiguous per partition.
    x_d = nc.declare_dram_parameter("x", [C, BPC, C], f32, isOutput=False)
    msc_d = nc.declare_dram_parameter("msc", [C + 1, NH, C + 1], bf16, isOutput=False)
    nv_d = nc.declare_dram_parameter("nv", [C + 1, NH, C], bf16, isOutput=False)
    gamma_d = nc.declare_dram_parameter("gamma", [C, 1], f32, isOutput=False)
    beta_d = nc.declare_dram_parameter("beta", [C, 1], f32, isOutput=False)
    gmask_d = nc.declare_dram_parameter("gmask", [C, C], f32, isOutput=False)
    ones_d = nc.declare_dram_parameter("ones96", [S, S], bf16, isOutput=False)
    maskt_d = nc.declare_dram_parameter("maskT", [S, S], bf16, isOutput=False)
    iden_d = nc.declare_dram_parameter("iden", [C, C], bf16, isOutput=False)
    boe_d = nc.declare_dram_parameter("bo_eff", [1, C], bf16, isOutput=False)
    out_d = nc.declare_dram_parameter("out", [BPC, C, NFREE], bf16, isOutput=True)

    with tile.TileContext(nc) as tc:
        with (
            tc.tile_pool(name="const", bufs=1) as cpool,
            tc.tile_pool(name="work", bufs=2) as work,
            tc.tile_pool(name="psum", bufs=PS_BUFS, space="PSUM") as pp,
            tc.tile_pool(name="dram", bufs=1, space="DRAM") as dpool,
        ):
            # ---- constants ----
            msc_sb = cpool.tile([C + 1, NH, C + 1], bf16, name="msc_sb")
            nv_sb = cpool.tile([C + 1, NH, C], bf16, name="nv_sb")
            gamma_sb = cpool.tile([C, 1], f32, name="gamma_sb")
            beta_sb = cpool.tile([C, 1], f32, name="beta_sb")
            gmask_sb = cpool.tile([C, C], f32, name="gmask_sb")
            ones_sb = cpool.tile([S, S], bf16, name="ones_sb")
            maskt_sb = cpool.tile([S, S], bf16, name="maskt_sb")
            iden_sb = cpool.tile([C, C], bf16, name="iden_sb")
            bo_rep = cpool.tile([C, C], bf16, name="bo_rep")
            xnp_all = cpool.tile([C, BPC, C], bf16, name="xnp_all")
            nring = 5 if DEEP else 4
            xnt_ring = [
                cpool.tile([C + 1, C], bf16, name=f"xnt{i}")
                for i in range(nring)
            ]
            a_sb = cpool.tile([C, NFREE], bf16, name="a_sb")
            x_all = cpool.tile([C, BPC, C], f32, name="x_all")
            x2_all = cpool.tile([C, BPC, C], f32, name="x2_all")

            # x first (everything in phase 1a waits on it), then the
            # groupnorm-critical consts, then the attention consts
            if "1" in phases:
                nc.sync.dma_start(out=x_all, in_=x_d[:])
            nc.sync.dma_start(out=gamma_sb, in_=gamma_d[:])
            nc.sync.dma_start(out=beta_sb, in_=beta_d[:])
            nc.sync.dma_start(out=gmask_sb, in_=gmask_d[:])
            nc.sync.dma_start(out=iden_sb, in_=iden_d[:])
            nc.sync.dma_start(out=msc_sb, in_=msc_d[:])
            nc.sync.dma_start(out=nv_sb, in_=nv_d[:])
            nc.sync.dma_start(out=ones_sb, in_=ones_d[:])
            nc.sync.dma_start(out=maskt_sb, in_=maskt_d[:])
            nc.sync.dma_start(out=bo_rep, in_=boe_d[:].to_broadcast((C, C)))
            for t in xnt_ring:
                nc.vector.memset(t[C : C + 1, :], 1.0)

            # DRAM bounce buffer for the collective. Gathered in batch
            # halves (strided output views keep global j order) so the
            # first gather overlaps the second half of phase-1 compute.
            HB = BPC // 2  # 6
            a_loc = dpool.tile([BPC, S, C], bf16, name="a_loc")
            a_full = dpool.tile(
                [NCORES * BPC, S, C],
                bf16,
                name="a_full",
                addr_space="Local" if skip_collective else "Shared",
            )

            inv_n = 1.0 / (C * C // G)  # 1/1152

            loop_cm = (
                tc.For_i(0, loop_n, 1)
                if loop_n > 1
                else contextlib.nullcontext()
            )
            loop_cm.__enter__()

            # ===== phase 1a: batched groupnorm stats for all 12 batches ====
            # x_all[p, b, w]; groups are 12-channel blocks of the partition
            # dim; per-(group,batch) stats via big row-reduces then a tiny
            # matmul through the group-averaging mask.
            if "1" in phases:
                nc.scalar.activation(
                    out=x2_all.rearrange("p b w -> p (b w)"),
                    in_=x_all.rearrange("p b w -> p (b w)"),
                    func=AF.Square,
                )
                rs = work.tile([C, 2, BPC], f32, tag="rs", bufs=1, name="rs")
                nc.vector.tensor_reduce(
                    out=rs[:, 0, :], in_=x_all, axis=AX.X, op=ALU.add
                )
                nc.vector.tensor_reduce(
                    out=rs[:, 1, :], in_=x2_all, axis=AX.X, op=ALU.add
                )
                psg = pp.tile([C, 2, BPC], f32, tag="ps", name="ps_g")
                nc.tensor.matmul(
                    psg.rearrange("p a b -> p (a b)"),
                    lhsT=gmask_sb,
                    rhs=rs.rearrange("p a b -> p (a b)"),
                    start=True,
                    stop=True,
                )
                mu = work.tile([C, BPC], f32, tag="gn", bufs=8, name="mu")
                ex2 = work.tile([C, BPC], f32, tag="gn", bufs=8, name="ex2")
                nc.vector.tensor_scalar_mul(mu, psg[:, 0, :], inv_n)
                nc.vector.tensor_scalar_mul(ex2, psg[:, 1, :], inv_n)
                musq = work.tile([C, BPC], f32, tag="gn", bufs=8, name="musq")
                nc.vector.tensor_mul(musq, mu, mu)
                veps = work.tile([C, BPC], f32, tag="gn", bufs=8, name="veps")
                nc.vector.scalar_tensor_tensor(
                    veps, ex2, EPS, musq, op0=ALU.add, op1=ALU.subtract
                )
                # rstd = rsqrt(veps), all-DVE (quake seed + 2 Newton steps) so
                # ACT only ever needs the exp_and_others table set.
                i32 = mybir.dt.int32
                iv = veps.bitcast(i32)
                ineg = work.tile([C, BPC], i32, tag="gni", bufs=2, name="ineg")
                nc.vector.tensor_scalar_mul(ineg, iv, -1)
                nc.vector.tensor_scalar(ineg, ineg, 1, None, op0=ALU.arith_shift_right)
                nc.vector.tensor_scalar(ineg, ineg, 0x5F3759DF, None, op0=ALU.add)
                y = ineg.bitcast(f32)
                t1 = work.tile([C, BPC], f32, tag="gn", bufs=8, name="t1")
                for _ in range(2):
                    nc.vector.tensor_mul(t1, y, y)
                    nc.vector.tensor_mul(t1, t1, veps)
                    nc.vector.tensor_scalar(t1, t1, -0.5, 1.5, op0=ALU.mult, op1=ALU.add)
                    nc.vector.tensor_mul(y, y, t1)
                scale_t = work.tile([C, BPC], f32, tag="gn", bufs=8, name="scale_t")
                nc.vector.tensor_mul(
                    scale_t, y, gamma_sb.to_broadcast((C, BPC))
                )
                mus = work.tile([C, BPC], f32, tag="gn", bufs=8, name="mus")
                nc.vector.tensor_mul(mus, mu, scale_t)
                shift_t = work.tile([C, BPC], f32, tag="gn", bufs=8, name="shift_t")
                nc.vector.tensor_sub(
                    shift_t, beta_sb.to_broadcast((C, BPC)), mus
                )

            # ===== phase 1b: per-batch attention, software-pipelined ========
            st = {}

            def st1(b):
                d = st[b] = {}
                xn_b = work.tile(
                    [C, C], bf16, tag="xn_b", bufs=4 if DEEP else 3, name="xn_b"
                )
                nc.vector.tensor_scalar(
                    xn_b,
                    x_all[:, b, :],
                    scale_t[:, b : b + 1],
                    shift_t[:, b : b + 1],
                    op0=ALU.mult,
                    op1=ALU.add,
                )
                nc.gpsimd.tensor_tensor(xnp_all[:, b, :], xn_b, bo_rep, ALU.add)
                ps_xt = pp.tile(
                    [C, C], bf16, tag="pxt", bufs=PXT_BUFS, name="ps_xt"
                )
                nc.tensor.transpose(ps_xt, xn_b, iden_sb)
                xnT = xnt_ring[b % nring]
                nc.vector.tensor_copy(out=xnT[0:C, :], in_=ps_xt)
                d["xnT"] = xnT

            def st2(b):
                d = st[b]
                xnT = d["xnT"]
                r_sb = work.tile(
                    [C + 1, NH, S],
                    bf16,
                    tag="r_sb",
                    bufs=5 if DEEP else 4,
                    name="r_sb",
                )
                for g in range(2):
                    psr = pp.tile([C + 1, 4 * S], f32, tag="ps", name="ps_r")
                    for hh in range(4):
                        h = 4 * g + hh
                        nc.tensor.matmul(
                            psr[:, hh * S : (hh + 1) * S],
                            lhsT=msc_sb[:, h, :],
                            rhs=xnT,
                            start=True,
                            stop=True,
                        )
                    r_slice = r_sb[:, 4 * g : 4 * (g + 1), :].rearrange(
                        "p h s -> p (h s)"
                    )
                    if g == 0 or R2ACT:
                        nc.scalar.copy(out=r_slice, in_=psr)
                    else:
                        nc.vector.tensor_copy(out=r_slice, in_=psr)
                d["r"] = r_sb

            def st3(b):
                d = st[b]
                xnT = d["xnT"]
                expT = work.tile(
                    [S, NH, S], bf16, tag="expT", bufs=5 if DEEP else 4, name="expT"
                )
                vw_sb = work.tile(
                    [S, NH, C], bf16, tag="vw_sb", bufs=5 if DEEP else 4, name="vw_sb"
                )
                for g in range(2):
                    pst = pp.tile([S, 4 * S], f32, tag="ps", name="ps_sc")
                    # one 384-wide matmul per 4-head pack: the head index
                    # rides the rhs free dim since lhsT (xnT) is shared
                    nc.tensor.matmul(
                        pst,
                        lhsT=xnT,
                        rhs=d["r"][:, 4 * g : 4 * (g + 1), :].rearrange(
                            "p h s -> p (h s)"
                        ),
                        start=True,
                        stop=True,
                    )
                    nc.scalar.activation(
                        out=expT[:, 4 * g : 4 * (g + 1), :].rearrange(
                            "p h s -> p (h s)"
                        ),
                        in_=pst,
                        func=AF.Exp,
                    )
                for g in range(2):
                    psv = pp.tile([S, 4 * C], f32, tag="ps", name="ps_v")
                    nc.tensor.matmul(
                        psv,
                        lhsT=xnT,
                        rhs=nv_sb[:, 4 * g : 4 * (g + 1), :].rearrange(
                            "p h s -> p (h s)"
                        ),
                        start=True,
                        stop=True,
                    )
                    vw_slice = vw_sb[:, 4 * g : 4 * (g + 1), :].rearrange(
                        "p h s -> p (h s)"
                    )
                    if g == 0:
                        nc.vector.tensor_copy(out=vw_slice, in_=psv)
                    else:
                        nc.scalar.copy(out=vw_slice, in_=psv)
                # mask while denominators are still pending
                meng = nc.gpsimd if POOL_MASK else nc.vector
                meng.tensor_tensor(
                    expT,
                    expT,
                    maskt_sb.unsqueeze(1).to_broadcast((S, NH, S)),
                    ALU.mult,
                )
                d["expT"], d["vw"] = expT, vw_sb

            def st4(b):
                d = st[b]
                expT = d["expT"]
                recip = work.tile(
                    [S, NH, S], bf16, tag="recip", bufs=4 if DEEP else 3, name="recip"
                )
                for g in range(2):
                    psd = pp.tile([S, 4 * S], f32, tag="ps", name="ps_d")
                    nc.tensor.matmul(
                        psd,
                        lhsT=ones_sb,
                        rhs=expT[:, 4 * g : 4 * (g + 1), :].rearrange(
                            "p h s -> p (h s)"
                        ),
                        start=True,
                        stop=True,
                    )
                    with nc.allow_low_precision("bf16 softmax denominators"):
                        nc.vector.reciprocal(
                            out=recip[:, 4 * g : 4 * (g + 1), :].rearrange(
                                "p h s -> p (h s)"
                            ),
                            in_=psd,
                        )
                aeng = nc.gpsimd if POOL_ATTN else nc.vector
                aeng.tensor_tensor(expT, expT, recip, ALU.mult)

            def st5(b):
                d = st.pop(b)
                psw = pp.tile([S, C], f32, tag="psw", bufs=2, name="ps_w")
                for h in range(NH):
                    nc.tensor.matmul(
                        psw,
                        lhsT=d["expT"][:, h, :],
                        rhs=d["vw"][:, h, :],
                        start=(h == 0),
                        stop=(h == NH - 1),
                    )
                outp_sb = work.tile(
                    [S, C], bf16, tag="outp_sb", bufs=3 if DEEP else 2, name="outp_sb"
                )
                nc.scalar.copy(out=outp_sb, in_=psw)
                {"gpsimd": nc.gpsimd, "scalar": nc.scalar, "sync": nc.sync}[
                    ALOC_ENG
                ].dma_start(out=a_loc[b], in_=outp_sb)

            def pe_fill():
                # dependency-free matmuls into a scratch bank: keep the PE
                # busy through stage gaps so DVFS stays at full clock
                psf = pp.tile([64, 512], f32, tag="fill", bufs=1, name="ps_f")
                nc.tensor.matmul(
                    psf,
                    lhsT=ones_sb[:, 0:64],
                    rhs=msc_sb[0 : S].rearrange("p h c -> p (h c)")[:, 0:512],
                    start=True,
                    stop=True,
                )

            chunks = (
                [(0, 9), (9, 12)]
                if ASYM
                else [(QB * q, QB * (q + 1)) for q in range(BPC // QB)]
            )

            def emit_gather(q):
                """Gather batch-chunk q of every core's A into the matching
                a_full rows. Emitted right after the producing pipeline
                group so all but the last chunk overlap remaining phase-1
                compute."""
                lo, hi = chunks[q]
                if skip_collective:
                    # timeline-sim variant: equivalent-traffic local DMAs;
                    # the exposed (last) chunk can spread across DMA rings
                    last = q == len(chunks) - 1
                    spread = GATHER_SPREAD == 2 or (GATHER_SPREAD and last)
                    for cc in range(NCORES):
                        qe = (
                            (nc.sync, nc.scalar, nc.gpsimd)[cc % 3]
                            if spread
                            else nc.sync
                        )
                        qe.dma_start(
                            out=a_full[
                                BPC * cc + lo : BPC * cc + hi
                            ].rearrange("b k l -> b (k l)"),
                            in_=a_loc[lo:hi].rearrange("b k l -> b (k l)"),
                        )
                elif q == len(chunks) - 1:
                    # the simulator (and possibly the runtime) requires a
                    # single writer for Shared DRAM, so the real path runs
                    # one full gather; only the bench substitute is chunked
                    # to overlap phase-1 compute.
                    nc.gpsimd.collective_compute(
                        "AllGather",
                        mybir.AluOpType.bypass,
                        replica_groups=[list(range(NCORES))],
                        ins=[a_loc.opt()],
                        outs=[a_full.opt()],
                    )

            def emit_loads():
                # wide k-chunk loads; first-chunk assembly overlaps the rest
                a_flat = a_full[:].rearrange("j k l -> j (k l)")
                nq = 4 if LOADS_Q else 2
                w = NFREE // nq
                for qq in range(nq):
                    if LOADS_Q:
                        qe = (nc.sync, nc.scalar)[qq % 2]
                    elif LOAD2_SCALAR and qq == 1:
                        qe = nc.scalar
                    else:
                        qe = nc.sync
                    qe.dma_start(
                        out=a_sb[:, qq * w : (qq + 1) * w],
                        in_=a_flat[:, qq * w : (qq + 1) * w],
                    )

            # Triple-interleaved emission: three batches advance stage-by-
            # stage together so every engine's in-order stream alternates
            # between independent batches, hiding cross-engine hop latency.
            # The batch-half-0 gather fires as soon as batches 0-5 are done
            # (phase 2 interleaved under phase-1 compute).
            if "1" in phases:
                emitted = 0
                for b0 in range(0, BPC, ILV):
                    for fn in (st1, st2, st3, st4, st5):
                        for bi in range(ILV):
                            fn(b0 + bi)
                        for _ in range(PEFILL):
                            pe_fill()
                    if "2" in phases or "a" in phases:
                        while (
                            emitted < len(chunks)
                            and chunks[emitted][1] <= b0 + ILV
                        ):
                            emit_gather(emitted)
                            emitted += 1
                if "2" in phases or "b" in phases:
                    emit_loads()
            elif "2" in phases or "a" in phases or "b" in phases:
                if "2" in phases or "a" in phases:
                    for q in range(len(chunks)):
                        emit_gather(q)
                if "2" in phases or "b" in phases:
                    emit_loads()

            a_3d = a_sb.rearrange("p (k l) -> p k l", l=C)

            # ================= phase 3: assemble + write output ============
            # half-slabs interleaved between DVE and GpSimd so both engine
            # streams run concurrently against the output DMA.
            KH = S // 2  # 48 k-rows per half-slab
            nhalf = 1 if FULL_SLAB else 2
            WID = NFREE // nhalf
            KW = S // nhalf
            for i in range(BPC) if "3" in phases else []:
                for half in range(nhalf):
                    g = i * nhalf + half
                    res_t = work.tile([C, WID], bf16, tag="res", bufs=RES_BUFS)
                    use_pool = (not FULL_SLAB) and g in _GPSIMD_HALVES
                    if FULL_SLAB and i % 6 == 3 and i // 6 < POOL_IS:
                        # unload DVE: this row's adds run as two GpSimd halves
                        for hh in range(2):
                            nc.gpsimd.tensor_tensor(
                                res_t.rearrange("p (k l) -> p k l", l=C)[
                                    :, hh * KH : (hh + 1) * KH, :
                                ],
                                a_3d[:, hh * KH : (hh + 1) * KH, :],
                                xnp_all[:, i, :]
                                .unsqueeze(1)
                                .to_broadcast((C, KH, C)),
                                mybir.AluOpType.add,
                            )
                    else:
                        eng = nc.gpsimd if use_pool else nc.vector
                        eng.tensor_tensor(
                            res_t.rearrange("p (k l) -> p k l", l=C),
                            a_3d[:, half * KW : (half + 1) * KW, :],
                            xnp_all[:, i, :]
                            .unsqueeze(1)
                            .to_broadcast((C, KW, C)),
                            mybir.AluOpType.add,
                        )
                    # round-robin the store queue: different issuing engines
                    # use different DMA rings, which can overlap on hardware
                    if OUT_SPREAD == 1:
                        qeng = (nc.sync, nc.scalar, nc.gpsimd)[
                            (g + OUT_ROT) % 3
                        ]
                    elif OUT_SPREAD == 2:
                        qeng = (nc.sync, nc.scalar)[g % 2]
                    else:
                        qeng = nc.sync
                    qeng.dma_start(
                        out=out_d[i][:, half * WID : (half + 1) * WID],
                        in_=res_t,
                    )

            loop_cm.__exit__(None, None, None)

    nc.compile()
    return nc


def _get_program():
    global _PROG
    if _PROG is None:
        _PROG = _build_program()
    return _PROG


def _host_inputs(x, Wq, bq, Wk, bk, Wv, bv, Wo, bo, gamma, beta):
    import ml_dtypes

    f32 = np.float32
    f64 = np.float64
    bf16 = ml_dtypes.bfloat16
    x = np.asarray(x, f32)
    gamma = np.asarray(gamma, f32)
    beta = np.asarray(beta, f32)

    wq3 = np.asarray(Wq, f64).reshape(C, NH, DK)
    bq3 = np.asarray(bq, f64).reshape(1, NH, DK)
    wk3 = np.asarray(Wk, f64).reshape(C, NH, DK)
    bk3 = np.asarray(bk, f64).reshape(1, NH, DK)
    wv3 = np.asarray(Wv, f64).reshape(C, NH, DK)
    bv3 = np.asarray(bv, f64).reshape(1, NH, DK)
    wo3 = np.asarray(Wo, f64).reshape(NH, DK, C)
    wqe = np.concatenate([wq3, bq3], axis=0)  # (97, NH, DK)
    wke = np.concatenate([wk3, bk3], axis=0)
    wve = np.concatenate([wv3, bv3], axis=0)
    msc = np.einsum("ahd,bhd->ahb", wqe, wke) / np.sqrt(f64(DK))  # (97,NH,97)
    nv = np.einsum("ahd,hdc->ahc", wve, wo3)  # (97, NH, C)

    com = {
        "msc": np.ascontiguousarray(msc.astype(bf16)),
        "nv": np.ascontiguousarray(nv.astype(bf16)),
        "gamma": np.ascontiguousarray(gamma.reshape(C, 1)),
        "beta": np.ascontiguousarray(beta.reshape(C, 1)),
        "gmask": np.kron(np.eye(G, dtype=f32), np.ones((C // G, C // G), f32)),
        "ones96": np.ones((S, S), bf16),
        "maskT": np.triu(np.ones((S, S), f32)).astype(bf16),
        "iden": np.eye(C, dtype=f32).astype(bf16),
        "bo_eff": np.asarray(bo, f32).reshape(1, C).astype(bf16),
    }
    x_r = x.reshape(B_TOTAL, C, C)
    in_maps = []
    for i in range(NCORES):
        m = dict(com)
        m["x"] = np.ascontiguousarray(
            x_r[i * BPC : (i + 1) * BPC].transpose(1, 0, 2)
        )
        in_maps.append(m)
    return in_maps


def _run(inputs, trace=False):
    from concourse.bass_utils import run_bass_kernel_spmd

    nc = _get_program()
    in_maps = _host_inputs(**inputs)
    res = run_bass_kernel_spmd(
        nc, in_maps, core_ids=list(range(NCORES)), trace=trace
    )
    out = np.concatenate([r["out"] for r in res.results], axis=0)
    return out.reshape(B_TOTAL, C, S, C).astype(np.float32), res


def kernel(**inputs) -> np.ndarray:
    out, _ = _run(inputs, trace=False)
    return out
